# revision 19
# baseline (speedup 1.0000x reference)
"""Multi-head attention (B=4, S=2048, D=1024, H=16, DH=64) on 8 TRN2 cores, v3.

Sharding: core c = (batch b, head-group g2) with b = c//2, g2 = c%2 —
each core: one batch, 8 heads, zero collectives.

v3 changes vs v2 (calibrated: PE ~110us is the bf16 roofline here; these
shave instruction overheads + free DVE/DMA slack):
  - output bf16 (host upcasts to f32): halves output DMA (4MB -> 2MB).
  - V projection moving widened to 256 (4 heads/matmul): half the MM +
    eviction instruction count for the same columns.
  (tried + reverted: normalizing from ctx PSUM directly to skip the sb
  copy — holds the psctx bank across ~10 DVE ops and stalls the next
  block's ctx matmuls; measured 110us -> 151us.)

v2 changes vs v1:
  - nkv granularity 128 (1152 typical vs 1536): K/V projections shrink ~20%.
  - context matmul flipped to out[q, d]: stationary = probs chunk
    [128 kv, 128 q] bf16, moving = v'+ones [128 kv, 65] bf16 -> PSUM
    [128 q, 65] accumulated over kv chunks. The ones column computes the
    softmax denominator per q PARTITION for +1 streamed column (v1's
    [d, q] orientation paid a full second instruction stream for it).
  - normalization fused on the idle GPSIMD engine via normalize_recip
    (out[i,j] = in[i,j]/denom[i]); PE broadcast matmul and the DVE
    copy/reciprocal/multiply chain are gone.
  - output written in natural [S, HD] orientation (no host transpose).
  - software pipeline: SC(i+1) emitted before CTX(i) so ACT exp of a
    block overlaps PE context work of the previous block; K/V/Q(c0)
    projections up front in a scoped pool (xk/wk/wv SBUF freed after),
    Q(c) emitted just-in-time one chunk ahead.

All matmul operands bf16 (halves input DMA + SBUF; rel err ~5e-3 vs
the 2e-2 gate); PSUM accumulation stays f32.
"""

import os
import sys

import numpy as np

sys.path.insert(0, "/opt/trn_rl_repo")

B, S, D = 4, 2048, 1024
H, DH = 16, 64
HPC = 8            # heads per core
HD = HPC * DH      # 512 output columns per core
NCORES = 8
KD = D // 128      # 8 contraction chunks
NT = HD // 128     # 4 head-dim partition chunks (= head pairs)
NC4 = S // 512     # 4 q chunks of 512
VW = DH + 1        # 65 v cols per head incl ones

_CACHED = {}


def _pieces(n):
    """Split n into kv pieces, each a multiple of 128 and >= 256 (f32r
    needs a moving dim >= 256 for full rate)."""
    out, off = [], 0
    while n - off > 768:
        out.append((off, 512))
        off += 512
    rem = n - off
    if rem > 512:
        out.append((off, rem - 256))
        out.append((off + rem - 256, 256))
    elif rem:
        out.append((off, rem))
    assert all(w >= 256 and w % 128 == 0 for _, w in out), out
    # smallest piece first: the first attention block's scores start
    # after the smallest possible xk transfer + kproj chunk
    widths = sorted(w for _, w in out)
    out = []
    off = 0
    for w in widths:
        out.append((off, w))
        off += w
    assert off == n
    return out


def _build_nc(nkv, nmk_attn=None, reps=1, has_bv=True):
    from concourse import bacc, mybir, tile

    f32 = mybir.dt.float32
    f32r = mybir.dt.float32r
    bf16 = mybir.dt.bfloat16
    i32 = mybir.dt.int32
    EXP = mybir.ActivationFunctionType.Exp
    MULT = mybir.AluOpType.mult
    ADD = mybir.AluOpType.add

    NMK = nkv // 128
    if nmk_attn is None:
        nmk_attn = NMK
    assert nmk_attn <= NMK
    kv_pieces = _pieces(nkv)

    nc = bacc.Bacc("TRN2", target_bir_lowering=False, debug=False,
                   enable_asserts=False)

    xt_d = nc.declare_dram_parameter("xt", [D, S], bf16, isOutput=False)
    xkt_d = nc.declare_dram_parameter("xkt", [D, nkv], bf16, isOutput=False)
    wq_d = nc.declare_dram_parameter("wq", [D, HD], bf16, isOutput=False)
    wk_d = nc.declare_dram_parameter("wk", [D, HD], bf16, isOutput=False)
    wv_d = nc.declare_dram_parameter("wv", [D, HD], bf16, isOutput=False)
    bq_d = nc.declare_dram_parameter("bq", [HD], f32, isOutput=False)
    bk_d = nc.declare_dram_parameter("bk", [HD], f32, isOutput=False)
    bv_d = (nc.declare_dram_parameter("bv", [HD], bf16, isOutput=False)
            if has_bv else None)
    mask_d = nc.declare_dram_parameter("maskc", [nkv], i32, isOutput=False)
    out_d = nc.declare_dram_parameter("out", [S, HD], bf16, isOutput=True)

    with tile.TileContext(nc) as tc:
        with (
            tc.tile_pool(name="const", bufs=1) as cpool,
            tc.tile_pool(name="qk", bufs=1) as qkpool,
            tc.tile_pool(name="vv", bufs=1) as vpool,
            tc.tile_pool(name="wq", bufs=1) as wqpool,
            tc.tile_pool(name="xt", bufs=4) as xpool,
            tc.tile_pool(name="probs", bufs=30) as ppool,
            tc.tile_pool(name="sbev", bufs=3) as sbpool,
            tc.tile_pool(name="outp", bufs=4) as opool,
            tc.tile_pool(name="psq", bufs=2, space="PSUM") as psq,
            tc.tile_pool(name="pssc", bufs=2, space="PSUM") as pssc,
            tc.tile_pool(name="psctx", bufs=1, space="PSUM") as psctx,
        ):
            ones_f = cpool.tile([128, 128], f32)
            nc.vector.memset(ones_f[:], 1.0)
            ones_bf = cpool.tile([128, NMK * HPC], bf16)
            nc.vector.tensor_copy(ones_bf[:], ones_f[:, 0:NMK * HPC])
            ones_r = cpool.tile([1, 128], bf16)
            nc.vector.tensor_copy(ones_r[:], ones_f[0:1, :])
            # compacted mask -> additive exp bias (128, NMK):
            # adder[p, m] = (maskc[m*128+p] - 1) * 10000
            mask_t = cpool.tile([128, NMK], i32)
            nc.sync.dma_start(
                mask_t[:], mask_d.ap().rearrange("(m p) -> p m", p=128))
            maskf = cpool.tile([128, NMK], f32)
            nc.vector.tensor_copy(maskf[:], mask_t[:])
            adder = cpool.tile([128, NMK], f32)
            nc.vector.tensor_scalar(adder[:], maskf[:], 10000.0, -10000.0,
                                    MULT, ADD)

            bq_t = cpool.tile([128, NT], f32)
            nc.sync.dma_start(
                bq_t[:], bq_d.ap().rearrange("(t p) -> p t", p=128))
            bk_t = cpool.tile([128, NT], f32)
            nc.sync.dma_start(
                bk_t[:], bk_d.ap().rearrange("(t p) -> p t", p=128))
            if has_bv:
                bv_r = cpool.tile([1, HD], bf16)
                nc.sync.dma_start(bv_r[:],
                                  bv_d.ap().rearrange("(o n) -> o n", o=1))

            qT = qkpool.tile([128, NT * S], bf16)
            kT = qkpool.tile([128, NT * nkv], bf16)
            # v' bf16: [128 kv, (m, h, 65)] with ones col at 64
            vB = vpool.tile([128, NMK * HPC * VW], bf16)
            nc.vector.tensor_copy(
                vB[:].rearrange("p (m h e) -> p m h e", m=NMK, h=HPC)
                [:, :, :, DH:DH + 1],
                ones_bf[:].rearrange("p (m h e) -> p m h e", m=NMK, h=HPC))
            wqt = wqpool.tile([128, KD * HD], bf16)

            xts = {}

            def fetch_xt(c):
                t_ = xpool.tile([128, KD * 512], bf16, tag="xt",
                                name=f"xt_{c}")
                (nc.sync if c == 0 else nc.gpsimd).dma_start(
                    t_[:], xt_d.ap()[:, c * 512:(c + 1) * 512].rearrange(
                        "(b p) s -> p b s", p=128))
                xts[c] = t_

            def scores_block(g, c, m_lo=0, m_hi=None):
                pr = []
                for m in range(m_lo, nmk_attn if m_hi is None else m_hi):
                    sc = pssc.tile([128, 1024], f32, tag="sc")
                    nc.tensor.matmul(
                        sc[:, 0:512],
                        kT[0:64, g * nkv + m * 128:
                           g * nkv + (m + 1) * 128],
                        qT[0:64, g * S + c * 512:
                           g * S + (c + 1) * 512],
                        start=True, stop=True)
                    nc.tensor.matmul(
                        sc[:, 512:1024],
                        kT[64:128, g * nkv + m * 128:
                           g * nkv + (m + 1) * 128],
                        qT[64:128, g * S + c * 512:
                           g * S + (c + 1) * 512],
                        start=True, stop=True)
                    probs = ppool.tile([128, 1024], bf16, tag="probs")
                    nc.scalar.activation(
                        probs[:], sc[:], EXP,
                        bias=adder[:, m:m + 1], scale=0.125)
                    pr.append(probs)
                return pr

            def ctx_block(g, c, pr):
                # group-major: each (qc, head) accumulation group is 9
                # consecutive matmuls into its own psum region. Each
                # ctx-tile half evicts/normalizes/DMAs as soon as its 4
                # groups finish, overlapping the other half's matmuls.
                # NOTE: the single full-width PSUM->SBUF copy is load-
                # bearing: it frees the psctx bank (bufs=1) after one DVE
                # instruction. Normalizing from PSUM directly holds the
                # bank across ~10 DVE ops and stalls the next block's ctx
                # matmuls (measured: 110us -> 151us body).
                ctxA = psctx.tile([128, 2 * VW * 2], f32, tag="ctxA")
                ctxB = psctx.tile([128, 2 * VW * 2], f32, tag="ctxB")
                sb = sbpool.tile([128, 4 * 2 * VW], f32, tag="sb")
                rc = sbpool.tile([128, 8], f32, tag="rc")
                o = opool.tile([128, 512], bf16, tag="o")
                for half, ctx in ((0, ctxA), (1, ctxB)):
                    hb = half * 2 * VW * 2
                    for qc in (2 * half, 2 * half + 1):
                        for hh in range(2):
                            base = (qc % 2) * 2 * VW + hh * VW
                            h = 2 * g + hh
                            for m in range(nmk_attn):
                                nc.tensor.matmul(
                                    ctx[:, base:base + VW],
                                    pr[m][:, hh * 512 + qc * 128:
                                          hh * 512 + (qc + 1) * 128],
                                    vB[:, m * HPC * VW + h * VW:
                                       m * HPC * VW + (h + 1) * VW],
                                    start=(m == 0),
                                    stop=(m == nmk_attn - 1))
                    nc.vector.tensor_copy(sb[:, hb:hb + 2 * VW * 2],
                                          ctx[:])
                    nc.vector.reciprocal_approx_fast(
                        out=rc[:, half * 4:(half + 1) * 4],
                        in_=sb[:, hb:hb + 2 * VW * 2]
                        .rearrange("p (s e) -> p s e", s=4)
                        [:, :, DH:DH + 1].rearrange("p s e -> p (s e)"))
                    for qc in (2 * half, 2 * half + 1):
                        for hh in range(2):
                            base = qc * 2 * VW + hh * VW
                            j = qc * 2 + hh
                            nc.vector.tensor_scalar_mul(
                                o[:, qc * 128 + hh * DH:
                                  qc * 128 + (hh + 1) * DH],
                                sb[:, base:base + DH],
                                rc[:, j:j + 1])
                    nc.sync.dma_start(
                        out_d.ap()[c * 512 + half * 256:
                                   c * 512 + (half + 1) * 256,
                                   2 * g * DH:(2 * g + 2) * DH].rearrange(
                            "(b p) h -> p b h", p=128),
                        o[:, half * 256:(half + 1) * 256])

            blocks = [(g, c) for c in range(NC4) for g in range(NT)]

            for rep in range(reps):
                with (
                    tc.tile_pool(name=f"xk{rep}", bufs=1) as xkpool,
                    tc.tile_pool(name=f"wkv{rep}", bufs=1) as wpool,
                ):
                    wkt = wpool.tile([128, KD * HD], bf16)
                    wvt = wpool.tile([128, KD * HD], bf16)
                    xk_t = xkpool.tile([128, KD * nkv], bf16,
                                       name="xk_t")

                    # DMA front: one big d-major transfer per
                    # tensor (issue cost dominates with many small DMAs)
                    xk3 = xkt_d.ap().rearrange("(b p) s -> p b s",
                                               p=128)
                    xkv = xk_t[:].rearrange("p (b s) -> p b s", b=KD)
                    fetch_xt(0)
                    xt0 = xts[0]
                    wq3 = wq_d.ap().rearrange("(b p) h -> p b h", p=128)
                    nc.sync.dma_start(
                        wqt[:].rearrange("p (b h) -> p b h", b=KD)
                        [:, :, 0:128], wq3[:, :, 0:128])
                    nc.sync.dma_start(
                        wkt[:], wk_d.ap().rearrange(
                            "(b p) h -> p b h", p=128))
                    for off, w in kv_pieces:
                        nc.gpsimd.dma_start(xkv[:, :, off:off + w],
                                            xk3[:, :, off:off + w])
                    nc.gpsimd.dma_start(
                        wvt[:], wv_d.ap().rearrange(
                            "(b p) h -> p b h", p=128))
                    for c in range(1, NC4):
                        fetch_xt(c)
                    nc.gpsimd.dma_start(
                        wqt[:].rearrange("p (b h) -> p b h", b=KD)
                        [:, :, 128:HD], wq3[:, :, 128:HD])

                    def kproj(t, pieces=None):
                        for off, w in (pieces or kv_pieces):
                            ps = psq.tile([128, 512], f32, tag="psqkv")
                            for d in range(KD):
                                nc.tensor.matmul(
                                    ps[:, 0:w],
                                    wkt[:, d * HD + t * 128:
                                        d * HD + (t + 1) * 128],
                                    xk_t[:, d * nkv + off:
                                         d * nkv + off + w],
                                    start=(d == 0), stop=(d == KD - 1))
                            nc.vector.tensor_scalar_add(
                                kT[:, t * nkv + off:t * nkv + off + w],
                                ps[:, 0:w], bk_t[:, t:t + 1])

                    def vproj(gp, ms):
                        # 4 heads per matmul (moving N=256)
                        h0 = 4 * gp
                        for m in ms:
                            ps = psq.tile([128, 512], f32, tag="psqkv")
                            for d in range(KD):
                                nc.tensor.matmul(
                                    ps[:, 0:256],
                                    xk_t[:, d * nkv + m * 128:
                                         d * nkv + (m + 1) * 128],
                                    wvt[:, d * HD + h0 * DH:
                                        d * HD + (h0 + 4) * DH],
                                    start=(d == 0),
                                    stop=(not has_bv and d == KD - 1))
                            if has_bv:
                                nc.tensor.matmul(
                                    ps[:, 0:256], ones_r[:],
                                    bv_r[:, h0 * DH:(h0 + 4) * DH],
                                    start=False, stop=True)
                            nc.vector.tensor_copy(
                                vB[:, m * HPC * VW + h0 * VW:
                                   m * HPC * VW + (h0 + 4) * VW]
                                .rearrange("p (h e) -> p h e",
                                           h=4)[:, :, 0:DH],
                                ps[:, 0:256].rearrange(
                                    "p (h e) -> p h e", h=4))

                    def qproj(t, c, xtc):
                        ps = psq.tile([128, 512], f32, tag="psqkv")
                        for d in range(KD):
                            nc.tensor.matmul(
                                ps[:],
                                wqt[:, d * HD + t * 128:
                                    d * HD + (t + 1) * 128],
                                xtc[:, d * 512:(d + 1) * 512],
                                start=(d == 0), stop=(d == KD - 1))
                        nc.vector.tensor_scalar_add(
                            qT[:, t * S + c * 512:t * S + (c + 1) * 512],
                            ps[:], bq_t[:, t:t + 1])

                    # ---- schedule: g-major rows. Only kproj(g)
                    # gates row g (one per 4 blocks); vproj pieces spread
                    # across row 0; qproj per block; ctx lags one block.
                    # first block: emit kproj(0) per piece, scores
                    # for the m-chunks each piece unblocks right behind it
                    mb = [(off + w) // 128 for off, w in kv_pieces]
                    mh = (nmk_attn + 1) // 2
                    qproj(0, 0, xts[0])
                    kproj(0, [kv_pieces[0]])
                    pr = scores_block(0, 0, 0, min(mb[0], nmk_attn))
                    for pi in range(1, len(kv_pieces)):
                        kproj(0, [kv_pieces[pi]])
                        pr += scores_block(0, 0, min(mb[pi - 1], nmk_attn),
                                           min(mb[pi], nmk_attn))
                    vproj(0, range(0, mh))
                    prev = (0, 0, pr)
                    # vproj half-ranges keyed by block; gp=0 covers heads
                    # 0-3 (g 0,1), gp=1 heads 4-7 (g 2,3). Deadline: gp=0
                    # fully emitted before ctx_block(0, 0) at block (0, 1)
                    vhalf = {(0, 1): (0, 1), (0, 2): (1, 0), (0, 3): (1, 1)}
                    for g in range(NT):
                        for c in range(NC4):
                            if (g, c) == (0, 0):
                                continue
                            if c >= 1 and g + 1 < NT:
                                kproj(g + 1, [kv_pieces[c - 1]])
                            qproj(g, c, xts[c])
                            pr = scores_block(g, c)
                            if (g, c) in vhalf:
                                vg, half = vhalf[(g, c)]
                                vproj(vg, range(0, mh) if half == 0
                                      else range(mh, nmk_attn))
                            if prev is not None:
                                ctx_block(*prev)
                            prev = (g, c, pr)
                    ctx_block(*prev)

    nc.compile()
    return nc


def get_nc(nkv, nmk_attn, has_bv=True):
    key = (nkv, nmk_attn, has_bv)
    if key not in _CACHED:
        _CACHED[key] = _build_nc(nkv, nmk_attn, has_bv=has_bv)
    return _CACHED[key]


def make_in_maps(nkv, x, mask, wq, bq, wk, bk, wv, bv):
    import ml_dtypes

    bf = ml_dtypes.bfloat16
    x = np.ascontiguousarray(np.asarray(x, dtype=np.float32))
    mask = np.ascontiguousarray(np.asarray(mask, dtype=np.int32))
    wq = np.asarray(wq, dtype=np.float32).astype(bf)
    wk = np.asarray(wk, dtype=np.float32).astype(bf)
    wv = np.asarray(wv, dtype=np.float32).astype(bf)
    bq = np.asarray(bq, dtype=np.float32)
    bk = np.asarray(bk, dtype=np.float32)
    bv = np.asarray(bv, dtype=np.float32).astype(bf)
    idx = []
    for b in range(B):
        on = np.flatnonzero(mask[b] != 0)
        off = np.flatnonzero(mask[b] == 0)
        ib = np.concatenate([on, off])[:nkv]
        idx.append(ib)
    in_maps = []
    for c in range(NCORES):
        b, g = c // 2, c % 2
        cols = slice(g * HD, (g + 1) * HD)
        xtb = np.ascontiguousarray(x[b].T.astype(bf))
        in_maps.append({
            "xt": xtb,
            "xkt": np.ascontiguousarray(xtb[:, idx[b]]),
            "wq": np.ascontiguousarray(wq[:, cols]),
            "wk": np.ascontiguousarray(wk[:, cols]),
            "wv": np.ascontiguousarray(wv[:, cols]),
            "bq": np.ascontiguousarray(bq[cols]),
            "bk": np.ascontiguousarray(bk[cols]),
            "bv": np.ascontiguousarray(bv[cols]),
            "maskc": np.ascontiguousarray(mask[b][idx[b]]),
        })
    return in_maps


def assemble_out(results):
    out = np.empty((B, S, H * DH), dtype=np.float32)
    for c in range(NCORES):
        b, g = c // 2, c % 2
        out[b, :, g * HD:(g + 1) * HD] = np.asarray(
            results[c]["out"], dtype=np.float32)
    return out


def pick_nkv(mask):
    mask = np.asarray(mask)
    nb_max = int((mask != 0).sum(axis=1).max())
    nmk_attn = max(2, -(-nb_max // 128))
    nkv = min(nmk_attn * 128, S)
    return nkv, nmk_attn


def run(trace=False, **inputs):
    from concourse.bass_utils import run_bass_kernel_spmd

    nkv, nmk_attn = pick_nkv(inputs["mask"])
    has_bv = bool(np.any(np.asarray(inputs["bv"])))
    nc = get_nc(nkv, nmk_attn, has_bv)
    in_maps = make_in_maps(nkv, **inputs)
    if not has_bv:
        for m in in_maps:
            m.pop("bv", None)
    res = run_bass_kernel_spmd(nc, in_maps, core_ids=list(range(NCORES)),
                               trace=trace)
    return assemble_out(res.results), res


def kernel(**inputs):
    out, _ = run(trace=False, **inputs)
    return out

